# revision 46
# baseline (speedup 1.0000x reference)
"""Trainium2 Bass kernel for AttentiveMatchingLayer.

Math (per batch):
  na = l2n(a, -1); nb = l2n(b, -1)
  alpha[d,e] = sum_t nb[t,d] * na[t,e]          (D x D)
  s[e] = 1/sqrt(max(sum_d alpha[d,e]^2, EPS))   (column l2n of alpha)
  h[t,e] = sum_d nb[t,d] * alpha[d,e] * s[e]
  persp[t,p] = sum_e aw*hw  with aw = l2n(a*W[p]), hw = l2n(h*W[p])
             = (sum_e a*h*W2) / (sqrt(sum_e a^2*W2) * sqrt(sum_e h^2*W2))
  where W2 = W^2. Row-scalings of a and b cancel inside persp, so:
    - alpha = b^T @ (a * c) with c[t] = 1/(||a_t|| * ||b_t||)
    - h~ = b @ alpha (raw b) is h up to a positive row scale -> usable
      directly; the alpha-column scale s is folded into the W2 weights of
      the num matmul and the ACT scale of the h^2 pass.

Sharding: pure data parallel, batch 32 -> 4 per core across 8 cores.
"""

import sys

import numpy as np

for _p in ("/opt/trn_rl_repo", "/root/.axon_site/_ro/trn_rl_repo"):
    if _p not in sys.path:
        sys.path.append(_p)

import concourse.bacc as bacc
import concourse.bass as bass
import concourse.tile as tile
from concourse import mybir
from concourse.masks import make_identity

N_CORES = 8
B, T, D, P = 32, 256, 512, 20
BL = B // N_CORES  # batches per core
KT = T // 128  # 2 t-tiles
KD = D // 128  # 4 d-tiles
F32 = mybir.dt.float32
F32R = mybir.dt.float32r  # full-rate fp32 matmul streaming (vs 4x for fp32)
EPS = 1e-12


def _r(ap):
    return ap.bitcast(F32R)


def _f(ap):
    return ap.bitcast(F32)
AF = mybir.ActivationFunctionType
OP = mybir.AluOpType

TRACE = False
LAST_RESULT = None

_NCS = {}


def _emit(nc, a_ext, b_ext, w_ext, out_ext, reps=1):
    a_dram = a_ext[:, :, :]
    b_dram = b_ext[:, :, :]
    w_dram = w_ext[:, :]
    out_dram = out_ext[:, :, :]

    with tile.TileContext(nc) as tc:
        import contextlib

        with contextlib.ExitStack() as ctx:
            const = ctx.enter_context(tc.tile_pool(name="const", bufs=1))
            io = ctx.enter_context(tc.tile_pool(name="io", bufs=3))
            work = ctx.enter_context(tc.tile_pool(name="work", bufs=3))
            ps = ctx.enter_context(tc.tile_pool(name="ps", bufs=2, space="PSUM"))

            # ---- constants ----
            identf = const.tile([128, 128], F32)
            make_identity(nc, identf)
            ident = const.tile([128, 128], F32R)
            nc.vector.tensor_copy(ident, identf)
            ones = const.tile([128, 1], F32R)
            nc.vector.memset(_f(ones), 1.0)
            eps12 = const.tile([128, 1], F32)
            nc.vector.memset(eps12, EPS)
            eps24 = const.tile([128, 1], F32)
            nc.vector.memset(eps24, 1e-24)
            eps30 = const.tile([128, 1], F32)
            nc.vector.memset(eps30, 1e-30)

            # W2T[d, p] = W[p, d]^2 transposed
            w_sb = const.tile([P, D], F32)
            nc.sync.dma_start(out=w_sb, in_=w_dram)
            w2_sb = const.tile([P, D], F32R)
            nc.scalar.square(w2_sb, w_sb)
            w2t_sb = const.tile([128, KD, P], F32R)
            psum_w2t = ps.tile([128, KD * P], F32R, tag="sp")
            for m in range(KD):
                nc.tensor.transpose(
                    psum_w2t[:, m * P : (m + 1) * P],
                    w2_sb[:, m * 128 : (m + 1) * 128],
                    ident[:P, :P],
                )
            nc.scalar.copy(w2t_sb, psum_w2t.rearrange("p (m q) -> p m q", m=KD))

            def stage1(bi):
                    # ---- load a, b ----
                a_sb = io.tile([128, KT, D], F32R, tag="a")
                b_sb = io.tile([128, KT, D], F32R, tag="b")
                for k in range(KT):
                    nc.sync.dma_start(
                        out=a_sb[:, k, :],
                        in_=_r(a_dram[bi, k * 128 : (k + 1) * 128, :]),
                    )
                    nc.sync.dma_start(
                        out=b_sb[:, k, :],
                        in_=_r(b_dram[bi, k * 128 : (k + 1) * 128, :]),
                    )

                # ---- transposes: aT[d, t], bT[d, t] (PE) ----
                aT = work.tile([128, KD, T], F32R, tag="aT")
                bT = work.tile([128, KD, T], F32R, tag="bT")
                aTsq = work.tile([128, KD, T], F32R, tag="aTsq")
                bTsq = work.tile([128, KD, T], F32R, tag="bTsq")
                for src, dstv, sq in ((a_sb, aT, aTsq), (b_sb, bT, bTsq)):
                    for mm in range(KD // 2):
                        ptp = ps.tile([128, 512], F32R, tag="tp")
                        for mi in range(2):
                            m = 2 * mm + mi
                            for k in range(KT):
                                nc.tensor.transpose(
                                    ptp[
                                        :,
                                        mi * 256 + k * 128 : mi * 256
                                        + (k + 1) * 128,
                                    ],
                                    src[:, k, m * 128 : (m + 1) * 128],
                                    ident,
                                )
                        dst = dstv.rearrange("p m t -> p (m t)")[
                            :, mm * 512 : (mm + 1) * 512
                        ]
                        nc.vector.tensor_copy(dst, ptp)
                        sqdst = sq.rearrange("p m t -> p (m t)")[
                            :, mm * 512 : (mm + 1) * 512
                        ]
                        nc.gpsimd.tensor_mul(sqdst, dst, dst)

                # ---- c = 1/(|a_t| * |b_t|): row sumsq via PE ones-matmul
                # over the transposed squares, then transpose back ----
                pss_c = ps.tile([1, 2 * T], F32, tag="tp")
                for qi, sq in enumerate((aTsq, bTsq)):
                    for k in range(KD):
                        nc.tensor.matmul(
                            pss_c[:, qi * T : (qi + 1) * T],
                            lhsT=ones[:, :],
                            rhs=sq[:, k, :],
                            start=(k == 0),
                            stop=(k == KD - 1),
                        )
                sc_row = work.tile([1, 2 * T], F32, tag="sc_row")
                nc.scalar.copy(sc_row, pss_c)
                psum_c = ps.tile([128, 2 * KT], F32, tag="tp")
                for j in range(2 * KT):
                    nc.tensor.transpose(
                        psum_c[:, j : j + 1],
                        sc_row[:, j * 128 : (j + 1) * 128],
                        identf[:1, :1],
                    )
                sc_sb = work.tile([128, 2 * KT], F32, tag="sc_sb")
                nc.scalar.copy(sc_sb, psum_c)
                prod = work.tile([128, KT], F32, tag="prod")
                nc.vector.tensor_mul(
                    prod, sc_sb[:, 0:KT], sc_sb[:, KT : 2 * KT]
                )
                nc.scalar.activation(
                    out=prod, in_=prod, func=AF.Sqrt, bias=eps24
                )
                c_sb = work.tile([128, KT], F32, tag="c")
                nc.vector.reciprocal(c_sb, prod)

                # ---- ac = a * c ----
                ac = work.tile([128, KT, D], F32R, tag="ac")
                for k in range(KT):
                    nc.vector.tensor_scalar_mul(
                        ac[:, k, :], a_sb[:, k, :], c_sb[:, k : k + 1]
                    )

                # ---- alpha = b^T @ ac; colsumsq via ones-matmul ----
                alpha_sb = work.tile([128, KD, D], F32R, tag="alpha")
                ss_sb = work.tile([1, D], F32, tag="ss")
                psum_ss = ps.tile([1, D], F32, tag="sp")
                for m in range(KD):
                    pal = ps.tile([128, D], F32, tag="alpha")
                    for k in range(KT):
                        nc.tensor.matmul(
                            pal,
                            lhsT=b_sb[:, k, m * 128 : (m + 1) * 128],
                            rhs=ac[:, k, :],
                            start=(k == 0),
                            stop=(k == KT - 1),
                        )
                    if m % 2 == 0:
                        nc.vector.tensor_copy(alpha_sb[:, m, :], pal)
                    else:
                        nc.scalar.copy(alpha_sb[:, m, :], pal)
                    asq = work.tile([128, D], F32R, tag="asq")
                    nc.gpsimd.tensor_mul(
                        asq, alpha_sb[:, m, :], alpha_sb[:, m, :]
                    )
                    nc.tensor.matmul(
                        psum_ss,
                        lhsT=ones[:, :],
                        rhs=asq[:, :],
                        start=(m == 0),
                        stop=(m == KD - 1),
                    )

                # ---- s = 1/sqrt(colsumsq + eps) as [e,1] tiles ----
                sinv = work.tile([128, KD], F32, tag="sinv")
                nc.scalar.copy(ss_sb, psum_ss)
                psum_st = ps.tile([128, KD], F32, tag="sp")
                for m in range(KD):
                    nc.tensor.transpose(
                        psum_st[:, m : m + 1],
                        ss_sb[:, m * 128 : (m + 1) * 128],
                        identf[:1, :1],
                    )
                s_sq = work.tile([128, KD], F32, tag="s_sq")
                nc.scalar.activation(
                    out=s_sq, in_=psum_st, func=AF.Sqrt, bias=eps12
                )
                nc.vector.reciprocal(sinv, s_sq)

                # w2ts = s * W2T (weights for the num matmul)
                w2ts = work.tile([128, KD, P], F32R, tag="w2ts")
                for m in range(KD):
                    nc.vector.tensor_scalar_mul(
                        w2ts[:, m, :], w2t_sb[:, m, :], sinv[:, m : m + 1]
                    )

                return dict(aT=aT, bT=bT, aTsq=aTsq, alpha_sb=alpha_sb,
                            sinv=sinv, w2ts=w2ts)

            def stage2(bi, st):
                aT, bT, aTsq = st["aT"], st["bT"], st["aTsq"]
                alpha_sb, sinv, w2ts = st["alpha_sb"], st["sinv"], st["w2ts"]
                    # ---- hT~[e, t] = sum_d alpha[d, e-tile] * bT[d, t] ----
                # consumed from PSUM: ahT = aT*hT~ (DVE), hTsq = (s*hT~)^2
                ahT = work.tile([128, KD, T], F32R, tag="ahT")
                hTsq = work.tile([128, KD, T], F32R, tag="hTsq")
                for m in range(KD):
                    ph = ps.tile([128, T], F32, tag="h")
                    for k in range(KD):
                        nc.tensor.matmul(
                            ph,
                            lhsT=alpha_sb[:, k, m * 128 : (m + 1) * 128],
                            rhs=bT[:, k, :],
                            start=(k == 0),
                            stop=(k == KD - 1),
                        )
                    nc.vector.tensor_mul(ahT[:, m, :], aT[:, m, :], _r(ph))
                    nc.scalar.activation(
                        out=hTsq[:, m, :],
                        in_=ph,
                        func=AF.Square,
                        scale=sinv[:, m : m + 1],
                    )

                # ---- persp matmuls: da, dh, num  [P, T] ----
                pda = ps.tile([P, T], F32, tag="sp")
                for k in range(KD):
                    nc.tensor.matmul(
                        pda,
                        lhsT=w2t_sb[:, k, :],
                        rhs=aTsq[:, k, :],
                        start=(k == 0),
                        stop=(k == KD - 1),
                    )
                da_sb = work.tile([P, T], F32, tag="da")
                nc.scalar.copy(da_sb, pda)

                pdh = ps.tile([P, T], F32, tag="sp")
                for k in range(KD):
                    nc.tensor.matmul(
                        pdh,
                        lhsT=w2t_sb[:, k, :],
                        rhs=hTsq[:, k, :],
                        start=(k == 0),
                        stop=(k == KD - 1),
                    )
                sd = work.tile([P, T], F32, tag="sd")
                nc.vector.tensor_mul(sd, pdh, da_sb)
                nc.scalar.activation(
                    out=sd, in_=sd, func=AF.Sqrt, bias=eps30[:P]
                )
                rsd = work.tile([P, T], F32, tag="rsd")
                nc.vector.reciprocal(rsd, sd)

                pnum = ps.tile([P, T], F32, tag="sp")
                for k in range(KD):
                    nc.tensor.matmul(
                        pnum,
                        lhsT=w2ts[:, k, :],
                        rhs=ahT[:, k, :],
                        start=(k == 0),
                        stop=(k == KD - 1),
                    )
                pp_sb = work.tile([P, T], F32R, tag="pp")
                nc.vector.tensor_mul(pp_sb, pnum, rsd)

                # ---- transpose persp [P, T] -> [t, p] and store ----
                out_sb = io.tile([128, KT, P], F32, tag="out")
                pso = ps.tile([128, KT * P], F32R, tag="sp")
                for j in range(KT):
                    nc.tensor.transpose(
                        pso[:, j * P : (j + 1) * P],
                        pp_sb[:, j * 128 : (j + 1) * 128],
                        ident[:P, :P],
                    )
                nc.scalar.copy(
                    out_sb, pso.rearrange("p (k q) -> p k q", k=KT)
                )
                nc.sync.dma_start(
                    out=out_dram[bi].rearrange("(k p) q -> p k q", p=128),
                    in_=out_sb,
                )


            def batch_body():
                st = {0: stage1(0)}
                for bi in range(BL):
                    if bi + 1 < BL:
                        st[bi + 1] = stage1(bi + 1)
                    stage2(bi, st.pop(bi))

            if reps > 1:
                with tc.For_i(0, reps, 1):
                    batch_body()
            else:
                batch_body()


def _build(reps=1):
    nc = bacc.Bacc("TRN2", target_bir_lowering=False, debug=False)
    a_ext = nc.declare_dram_parameter("inp_a", [BL, T, D], F32, isOutput=False)
    b_ext = nc.declare_dram_parameter("inp_b", [BL, T, D], F32, isOutput=False)
    w_ext = nc.declare_dram_parameter("W", [P, D], F32, isOutput=False)
    out_ext = nc.declare_dram_parameter("persp", [BL, T, P], F32, isOutput=True)
    _emit(nc, a_ext, b_ext, w_ext, out_ext, reps=reps)
    nc.compile()
    return nc


def get_nc(reps=1):
    if reps not in _NCS:
        _NCS[reps] = _build(reps=reps)
    return _NCS[reps]


def run_on_cores(inp_a, inp_b, W, reps=1, trace=False):
    from concourse.bass_utils import run_bass_kernel_spmd

    nc = get_nc(reps)
    inp_a = np.ascontiguousarray(inp_a, dtype=np.float32)
    inp_b = np.ascontiguousarray(inp_b, dtype=np.float32)
    W = np.ascontiguousarray(W, dtype=np.float32)
    in_maps = [
        {
            "inp_a": inp_a[i * BL : (i + 1) * BL],
            "inp_b": inp_b[i * BL : (i + 1) * BL],
            "W": W,
        }
        for i in range(N_CORES)
    ]
    return run_bass_kernel_spmd(nc, in_maps, list(range(N_CORES)), trace=trace)


def kernel(inp_a, inp_b, W):
    global LAST_RESULT
    res = run_on_cores(inp_a, inp_b, W, reps=1, trace=TRACE)
    LAST_RESULT = res
    persp = np.concatenate(
        [res.results[i]["persp"] for i in range(N_CORES)], axis=0
    )
    return (persp, persp)


# revision 59
# speedup vs baseline: 1.0881x; 1.0881x over previous
"""Trainium2 Bass kernel for AttentiveMatchingLayer.

Math (per batch):
  na = l2n(a, -1); nb = l2n(b, -1)
  alpha[d,e] = sum_t nb[t,d] * na[t,e]          (D x D)
  s[e] = 1/sqrt(max(sum_d alpha[d,e]^2, EPS))   (column l2n of alpha)
  h[t,e] = sum_d nb[t,d] * alpha[d,e] * s[e]
  persp[t,p] = sum_e aw*hw  with aw = l2n(a*W[p]), hw = l2n(h*W[p])
             = (sum_e a*h*W2) / (sqrt(sum_e a^2*W2) * sqrt(sum_e h^2*W2))
  where W2 = W^2. Row-scalings of a and b cancel inside persp, so:
    - alpha = b^T @ (a * c) with c[t] = 1/(||a_t|| * ||b_t||)
    - h~ = b @ alpha (raw b) is h up to a positive row scale -> usable
      directly; the alpha-column scale s is folded into the W2 weights of
      the num matmul and the ACT scale of the h^2 pass.

Sharding: pure data parallel, batch 32 -> 4 per core across 8 cores.
"""

import sys

import numpy as np

for _p in ("/opt/trn_rl_repo", "/root/.axon_site/_ro/trn_rl_repo"):
    if _p not in sys.path:
        sys.path.append(_p)

import concourse.bacc as bacc
import concourse.bass as bass
import concourse.tile as tile
from concourse import mybir
from concourse.masks import make_identity

N_CORES = 8
B, T, D, P = 32, 256, 512, 20
BL = B // N_CORES  # batches per core
KT = T // 128  # 2 t-tiles
KD = D // 128  # 4 d-tiles
F32 = mybir.dt.float32
F32R = mybir.dt.float32r  # full-rate fp32 matmul streaming (vs 4x for fp32)
EPS = 1e-12


def _r(ap):
    return ap.bitcast(F32R)


def _f(ap):
    return ap.bitcast(F32)
AF = mybir.ActivationFunctionType
OP = mybir.AluOpType

TRACE = False
LAST_RESULT = None

_NCS = {}


def _emit(nc, a_ext, b_ext, w_ext, out_ext, reps=1):
    a_dram = a_ext[:, :, :]
    b_dram = b_ext[:, :, :]
    w_dram = w_ext[:, :]
    out_dram = out_ext[:, :, :]

    with tile.TileContext(nc) as tc:
        import contextlib

        with contextlib.ExitStack() as ctx:
            const = ctx.enter_context(tc.tile_pool(name="const", bufs=1))
            io = ctx.enter_context(tc.tile_pool(name="io", bufs=3))
            work = ctx.enter_context(tc.tile_pool(name="work", bufs=3))
            ps = ctx.enter_context(tc.tile_pool(name="ps", bufs=2, space="PSUM"))

            # ---- constants ----
            identf = const.tile([128, 128], F32)
            make_identity(nc, identf)
            ident = const.tile([128, 128], F32R)
            nc.vector.tensor_copy(ident, identf)
            ones = const.tile([128, 1], F32R)
            nc.vector.memset(_f(ones), 1.0)
            eps12 = const.tile([128, 1], F32)
            nc.vector.memset(eps12, EPS)
            eps24 = const.tile([128, 1], F32)
            nc.vector.memset(eps24, 1e-24)
            eps30 = const.tile([128, 1], F32)
            nc.vector.memset(eps30, 1e-30)

            # W2T[d, p] = W[p, d]^2 transposed
            w_sb = const.tile([P, D], F32)
            nc.sync.dma_start(out=w_sb, in_=w_dram)
            w2_sb = const.tile([P, D], F32R)
            nc.scalar.square(w2_sb, w_sb)
            w2t_sb = const.tile([128, KD, P], F32R)
            psum_w2t = ps.tile([128, KD * P], F32R, tag="sp")
            for m in range(KD):
                nc.tensor.transpose(
                    psum_w2t[:, m * P : (m + 1) * P],
                    w2_sb[:, m * 128 : (m + 1) * 128],
                    ident[:P, :P],
                )
            nc.scalar.copy(w2t_sb, psum_w2t.rearrange("p (m q) -> p m q", m=KD))

            def stage1(bi):
                    # ---- load a, b ----
                a_sb = io.tile([128, KT, D], F32R, tag="a")
                b_sb = io.tile([128, KT, D], F32R, tag="b")
                for k in range(KT):
                    nc.sync.dma_start(
                        out=a_sb[:, k, :],
                        in_=_r(a_dram[bi, k * 128 : (k + 1) * 128, :]),
                    )
                    nc.sync.dma_start(
                        out=b_sb[:, k, :],
                        in_=_r(b_dram[bi, k * 128 : (k + 1) * 128, :]),
                    )

                # ---- transposes: aT[d, t], bT[d, t] (PE) ----
                aT = work.tile([128, KD, T], F32R, tag="aT")
                bT = work.tile([128, KD, T], F32R, tag="bT")
                aTsq = work.tile([128, KD, T], F32R, tag="aTsq")
                bTsq = work.tile([128, KD, T], F32R, tag="bTsq")
                for si, (src, dstv, sq) in enumerate(
                    ((a_sb, aT, aTsq), (b_sb, bT, bTsq))
                ):
                    for k in range(KT):
                        ptp = ps.tile([128, 512], F32R, tag="tp")
                        for m in range(KD):
                            nc.tensor.transpose(
                                ptp[:, m * 128 : (m + 1) * 128],
                                src[:, k, m * 128 : (m + 1) * 128],
                                ident,
                            )
                        ptpv = ptp.rearrange("p (m q) -> p m q", m=KD)
                        dst = dstv[:, :, k * 128 : (k + 1) * 128]
                        if (si + k) % 2 == 0:
                            nc.vector.tensor_copy(dst, ptpv)
                        else:
                            nc.scalar.copy(dst, ptpv)
                        sqdst = sq[:, :, k * 128 : (k + 1) * 128]
                        nc.gpsimd.tensor_mul(sqdst, dst, dst)

                # ---- c = 1/(|a_t| * |b_t|) ----
                if bi == 0:
                    # fill-phase fast path: ACT square+accum straight off
                    # a_sb/b_sb, concurrent with the PE transposes
                    sa0 = work.tile([128, KT], F32, tag="sa0")
                    sb0 = work.tile([128, KT], F32, tag="sb0")
                    scr0 = work.tile([128, D], F32, tag="scr0")
                    for k in range(KT):
                        nc.scalar.activation(
                            out=scr0,
                            in_=_f(a_sb[:, k, :]),
                            func=AF.Square,
                            accum_out=sa0[:, k : k + 1],
                        )
                        nc.scalar.activation(
                            out=scr0,
                            in_=_f(b_sb[:, k, :]),
                            func=AF.Square,
                            accum_out=sb0[:, k : k + 1],
                        )
                    prod = work.tile([128, KT], F32, tag="prod")
                    nc.vector.tensor_mul(prod, sa0, sb0)
                    nc.scalar.activation(
                        out=prod, in_=prod, func=AF.Sqrt, bias=eps24
                    )
                    c_sb = work.tile([128, KT], F32, tag="c")
                    nc.vector.reciprocal(c_sb, prod)
                # row sumsq via PE ones-matmul over the transposed squares
                pss_c = ps.tile([1, 2 * T], F32, tag="tp")
                for qi, sq in enumerate((aTsq, bTsq)):
                    for k in range(KD):
                        nc.tensor.matmul(
                            pss_c[:, qi * T : (qi + 1) * T],
                            lhsT=ones[:, :],
                            rhs=sq[:, k, :],
                            start=(k == 0),
                            stop=(k == KD - 1),
                        )
                sc_row = work.tile([1, 2 * T], F32, tag="sc_row")
                nc.scalar.copy(sc_row, pss_c)
                psum_c = ps.tile([128, 2 * KT], F32, tag="tp")
                for j in range(2 * KT):
                    nc.tensor.transpose(
                        psum_c[:, j : j + 1],
                        sc_row[:, j * 128 : (j + 1) * 128],
                        identf[:1, :1],
                    )
                sc_sb = work.tile([128, 2 * KT], F32, tag="sc_sb")
                nc.scalar.copy(sc_sb, psum_c)
                prod = work.tile([128, KT], F32, tag="prod")
                nc.vector.tensor_mul(
                    prod, sc_sb[:, 0:KT], sc_sb[:, KT : 2 * KT]
                )
                nc.scalar.activation(
                    out=prod, in_=prod, func=AF.Sqrt, bias=eps24
                )
                c_sb = work.tile([128, KT], F32, tag="c")
                nc.vector.reciprocal(c_sb, prod)


                # ---- ac = a * c ----
                ac = work.tile([128, KT, D], F32R, tag="ac")
                for k in range(KT):
                    nc.gpsimd.tensor_scalar_mul(
                        ac[:, k, :], a_sb[:, k, :], c_sb[:, k : k + 1]
                    )

                # ---- alpha = b^T @ ac; colsumsq via ones-matmul ----
                alpha_sb = work.tile([128, KD, D], F32R, tag="alpha")
                ss_sb = work.tile([1, D], F32, tag="ss")
                psum_ss = ps.tile([1, D], F32, tag="sp")
                for m in range(KD):
                    pal = ps.tile([128, D], F32, tag="alpha")
                    for k in range(KT):
                        nc.tensor.matmul(
                            pal,
                            lhsT=b_sb[:, k, m * 128 : (m + 1) * 128],
                            rhs=ac[:, k, :],
                            start=(k == 0),
                            stop=(k == KT - 1),
                        )
                    if m % 2 == 0:
                        nc.vector.tensor_copy(alpha_sb[:, m, :], pal)
                    else:
                        nc.scalar.copy(alpha_sb[:, m, :], pal)
                    asq = work.tile([128, D], F32R, tag="asq")
                    nc.gpsimd.tensor_mul(
                        asq, alpha_sb[:, m, :], alpha_sb[:, m, :]
                    )
                    nc.tensor.matmul(
                        psum_ss,
                        lhsT=ones[:, :],
                        rhs=asq[:, :],
                        start=(m == 0),
                        stop=(m == KD - 1),
                    )

                # ---- da = sum_e aT^2 * W2T, and rda = 1/sqrt(da) (early,
                # independent of hmean -> off the batch tail) ----
                pda = ps.tile([P, T], F32, tag="sp")
                for k in range(KD):
                    nc.tensor.matmul(
                        pda,
                        lhsT=w2t_sb[:, k, :],
                        rhs=aTsq[:, k, :],
                        start=(k == 0),
                        stop=(k == KD - 1),
                    )
                rda = work.tile([P, T], F32, tag="rda")
                nc.scalar.activation(
                    out=rda, in_=pda, func=AF.Sqrt, bias=eps30[:P]
                )
                nc.vector.reciprocal(rda, rda)

                # ---- s = 1/sqrt(colsumsq + eps) as [e,1] tiles ----
                sinv = work.tile([128, KD], F32, tag="sinv")
                nc.scalar.copy(ss_sb, psum_ss)
                psum_st = ps.tile([128, KD], F32, tag="sp")
                for m in range(KD):
                    nc.tensor.transpose(
                        psum_st[:, m : m + 1],
                        ss_sb[:, m * 128 : (m + 1) * 128],
                        identf[:1, :1],
                    )
                s_sq = work.tile([128, KD], F32, tag="s_sq")
                nc.scalar.activation(
                    out=s_sq, in_=psum_st, func=AF.Sqrt, bias=eps12
                )
                nc.vector.reciprocal(sinv, s_sq)

                # w2ts = s * W2T (weights for the num matmul)
                w2ts = work.tile([128, KD, P], F32R, tag="w2ts")
                for m in range(KD):
                    nc.gpsimd.tensor_scalar_mul(
                        w2ts[:, m, :], w2t_sb[:, m, :], sinv[:, m : m + 1]
                    )

                return dict(aT=aT, bT=bT, alpha_sb=alpha_sb, sinv=sinv,
                            w2ts=w2ts, rda=rda)

            def stage2(bi, st):
                aT, bT = st["aT"], st["bT"]
                alpha_sb, sinv, w2ts = st["alpha_sb"], st["sinv"], st["w2ts"]
                rda, w2t_loc = st["rda"], w2t_sb

                # ---- hT~[e, t] = sum_d alpha[d, e-tile] * bT[d, t] ----
                # consumed from PSUM: ahT = aT*hT~ (DVE), hTsq = (s*hT~)^2
                ahT = work.tile([128, KD, T], F32R, tag="ahT")
                hTsq = work.tile([128, KD, T], F32R, tag="hTsq")
                for m in range(KD):
                    ph = ps.tile([128, T], F32, tag="h")
                    for k in range(KD):
                        nc.tensor.matmul(
                            ph,
                            lhsT=alpha_sb[:, k, m * 128 : (m + 1) * 128],
                            rhs=bT[:, k, :],
                            start=(k == 0),
                            stop=(k == KD - 1),
                        )
                    nc.vector.tensor_mul(ahT[:, m, :], aT[:, m, :], _r(ph))
                    nc.scalar.activation(
                        out=hTsq[:, m, :],
                        in_=ph,
                        func=AF.Square,
                        scale=sinv[:, m : m + 1],
                    )

                # ---- dh and num matmuls, combine with rda/rdh ----
                pdh = ps.tile([P, T], F32, tag="h")
                for k in range(KD):
                    nc.tensor.matmul(
                        pdh,
                        lhsT=w2t_sb[:, k, :],
                        rhs=hTsq[:, k, :],
                        start=(k == 0),
                        stop=(k == KD - 1),
                    )
                rdh = work.tile([P, T], F32, tag="rdh")
                nc.scalar.activation(
                    out=rdh, in_=pdh, func=AF.Sqrt, bias=eps30[:P]
                )
                nc.vector.reciprocal(rdh, rdh)

                pnum = ps.tile([P, T], F32, tag="sp")
                for k in range(KD):
                    nc.tensor.matmul(
                        pnum,
                        lhsT=w2ts[:, k, :],
                        rhs=ahT[:, k, :],
                        start=(k == 0),
                        stop=(k == KD - 1),
                    )
                pp1 = work.tile([P, T], F32, tag="pp1")
                nc.vector.tensor_mul(pp1, pnum, rda)
                pp_sb = work.tile([P, T], F32R, tag="pp")
                nc.vector.tensor_mul(pp_sb, pp1, rdh)

                # ---- transpose persp [P, T] -> [t, p] and store ----
                out_sb = io.tile([128, KT, P], F32, tag="out")
                pso = ps.tile([128, KT * P], F32R, tag="sp")
                for j in range(KT):
                    nc.tensor.transpose(
                        pso[:, j * P : (j + 1) * P],
                        pp_sb[:, j * 128 : (j + 1) * 128],
                        ident[:P, :P],
                    )
                nc.scalar.copy(
                    out_sb, pso.rearrange("p (k q) -> p k q", k=KT)
                )
                nc.sync.dma_start(
                    out=out_dram[bi].rearrange("(k p) q -> p k q", p=128),
                    in_=out_sb,
                )

            def batch_body():
                st = {0: stage1(0)}
                for bi in range(BL):
                    if bi + 1 < BL:
                        st[bi + 1] = stage1(bi + 1)
                    stage2(bi, st.pop(bi))

            if reps > 1:
                with tc.For_i(0, reps, 1):
                    batch_body()
            else:
                batch_body()


def _build(reps=1):
    nc = bacc.Bacc("TRN2", target_bir_lowering=False, debug=False)
    a_ext = nc.declare_dram_parameter("inp_a", [BL, T, D], F32, isOutput=False)
    b_ext = nc.declare_dram_parameter("inp_b", [BL, T, D], F32, isOutput=False)
    w_ext = nc.declare_dram_parameter("W", [P, D], F32, isOutput=False)
    out_ext = nc.declare_dram_parameter("persp", [BL, T, P], F32, isOutput=True)
    _emit(nc, a_ext, b_ext, w_ext, out_ext, reps=reps)
    nc.compile()
    return nc


def get_nc(reps=1):
    if reps not in _NCS:
        _NCS[reps] = _build(reps=reps)
    return _NCS[reps]


def run_on_cores(inp_a, inp_b, W, reps=1, trace=False):
    from concourse.bass_utils import run_bass_kernel_spmd

    nc = get_nc(reps)
    inp_a = np.ascontiguousarray(inp_a, dtype=np.float32)
    inp_b = np.ascontiguousarray(inp_b, dtype=np.float32)
    W = np.ascontiguousarray(W, dtype=np.float32)
    in_maps = [
        {
            "inp_a": inp_a[i * BL : (i + 1) * BL],
            "inp_b": inp_b[i * BL : (i + 1) * BL],
            "W": W,
        }
        for i in range(N_CORES)
    ]
    return run_bass_kernel_spmd(nc, in_maps, list(range(N_CORES)), trace=trace)


def kernel(inp_a, inp_b, W):
    global LAST_RESULT
    res = run_on_cores(inp_a, inp_b, W, reps=1, trace=TRACE)
    LAST_RESULT = res
    persp = np.concatenate(
        [res.results[i]["persp"] for i in range(N_CORES)], axis=0
    )
    return (persp, persp)


# revision 62
# speedup vs baseline: 1.0944x; 1.0058x over previous
"""Trainium2 Bass kernel for AttentiveMatchingLayer.

Math (per batch):
  na = l2n(a, -1); nb = l2n(b, -1)
  alpha[d,e] = sum_t nb[t,d] * na[t,e]          (D x D)
  s[e] = 1/sqrt(max(sum_d alpha[d,e]^2, EPS))   (column l2n of alpha)
  h[t,e] = sum_d nb[t,d] * alpha[d,e] * s[e]
  persp[t,p] = sum_e aw*hw  with aw = l2n(a*W[p]), hw = l2n(h*W[p])
             = (sum_e a*h*W2) / (sqrt(sum_e a^2*W2) * sqrt(sum_e h^2*W2))
  where W2 = W^2. Row-scalings of a and b cancel inside persp, so:
    - alpha = b^T @ (a * c) with c[t] = 1/(||a_t|| * ||b_t||)
    - h~ = b @ alpha (raw b) is h up to a positive row scale -> usable
      directly; the alpha-column scale s is folded into the W2 weights of
      the num matmul and the ACT scale of the h^2 pass.

Sharding: pure data parallel, batch 32 -> 4 per core across 8 cores.
"""

import sys

import numpy as np

for _p in ("/opt/trn_rl_repo", "/root/.axon_site/_ro/trn_rl_repo"):
    if _p not in sys.path:
        sys.path.append(_p)

import concourse.bacc as bacc
import concourse.bass as bass
import concourse.tile as tile
from concourse import mybir
from concourse.masks import make_identity

N_CORES = 8
B, T, D, P = 32, 256, 512, 20
BL = B // N_CORES  # batches per core
KT = T // 128  # 2 t-tiles
KD = D // 128  # 4 d-tiles
F32 = mybir.dt.float32
F32R = mybir.dt.float32r  # full-rate fp32 matmul streaming (vs 4x for fp32)
EPS = 1e-12


def _r(ap):
    return ap.bitcast(F32R)


def _f(ap):
    return ap.bitcast(F32)
AF = mybir.ActivationFunctionType
OP = mybir.AluOpType

TRACE = False
LAST_RESULT = None

_NCS = {}


def _emit(nc, a_ext, b_ext, w_ext, out_ext, reps=1):
    a_dram = a_ext[:, :, :]
    b_dram = b_ext[:, :, :]
    w_dram = w_ext[:, :]
    out_dram = out_ext[:, :, :]

    with tile.TileContext(nc) as tc:
        import contextlib

        with contextlib.ExitStack() as ctx:
            const = ctx.enter_context(tc.tile_pool(name="const", bufs=1))
            io = ctx.enter_context(tc.tile_pool(name="io", bufs=3))
            work = ctx.enter_context(tc.tile_pool(name="work", bufs=3))
            ps = ctx.enter_context(tc.tile_pool(name="ps", bufs=2, space="PSUM"))

            # ---- constants ----
            identf = const.tile([128, 128], F32)
            make_identity(nc, identf)
            ident = const.tile([128, 128], F32R)
            nc.vector.tensor_copy(ident, identf)
            ones = const.tile([128, 1], F32R)
            nc.vector.memset(_f(ones), 1.0)
            eps12 = const.tile([128, 1], F32)
            nc.vector.memset(eps12, EPS)
            eps24 = const.tile([128, 1], F32)
            nc.vector.memset(eps24, 1e-24)
            eps30 = const.tile([128, 1], F32)
            nc.vector.memset(eps30, 1e-30)

            # W2T[d, p] = W[p, d]^2 transposed
            w_sb = const.tile([P, D], F32)
            nc.sync.dma_start(out=w_sb, in_=w_dram)
            w2_sb = const.tile([P, D], F32R)
            nc.scalar.square(w2_sb, w_sb)
            w2t_sb = const.tile([128, KD, P], F32R)
            psum_w2t = ps.tile([128, KD * P], F32R, tag="sp")
            for m in range(KD):
                nc.tensor.transpose(
                    psum_w2t[:, m * P : (m + 1) * P],
                    w2_sb[:, m * 128 : (m + 1) * 128],
                    ident[:P, :P],
                )
            nc.scalar.copy(w2t_sb, psum_w2t.rearrange("p (m q) -> p m q", m=KD))

            def stage1(bi):
                    # ---- load a, b ----
                a_sb = io.tile([128, KT, D], F32R, tag="a")
                b_sb = io.tile([128, KT, D], F32R, tag="b")
                for k in range(KT):
                    nc.sync.dma_start(
                        out=a_sb[:, k, :],
                        in_=_r(a_dram[bi, k * 128 : (k + 1) * 128, :]),
                    )
                    nc.sync.dma_start(
                        out=b_sb[:, k, :],
                        in_=_r(b_dram[bi, k * 128 : (k + 1) * 128, :]),
                    )

                # ---- transposes: aT[d, t], bT[d, t] (PE) ----
                aT = work.tile([128, KD, T], F32R, tag="aT")
                bT = work.tile([128, KD, T], F32R, tag="bT")
                aTsq = work.tile([128, KD, T], F32R, tag="aTsq")
                bTsq = work.tile([128, KD, T], F32R, tag="bTsq")
                for si, (src, dstv, sq) in enumerate(
                    ((a_sb, aT, aTsq), (b_sb, bT, bTsq))
                ):
                    for k in range(KT):
                        ptp = ps.tile([128, 512], F32R, tag="tp")
                        for m in range(KD):
                            nc.tensor.transpose(
                                ptp[:, m * 128 : (m + 1) * 128],
                                src[:, k, m * 128 : (m + 1) * 128],
                                ident,
                            )
                        ptpv = ptp.rearrange("p (m q) -> p m q", m=KD)
                        dst = dstv[:, :, k * 128 : (k + 1) * 128]
                        if (si + k) % 2 == 0:
                            nc.vector.tensor_copy(dst, ptpv)
                        else:
                            nc.scalar.copy(dst, ptpv)
                        sqdst = sq[:, :, k * 128 : (k + 1) * 128]
                        nc.gpsimd.tensor_mul(sqdst, dst, dst)

                # ---- c = 1/(|a_t| * |b_t|) ----
                if bi == 0:
                    # fill-phase fast path: ACT square+accum straight off
                    # a_sb/b_sb, concurrent with the PE transposes
                    sa0 = work.tile([128, KT], F32, tag="sa0")
                    sb0 = work.tile([128, KT], F32, tag="sb0")
                    scr0 = work.tile([128, D], F32, tag="scr0")
                    for k in range(KT):
                        nc.scalar.activation(
                            out=scr0,
                            in_=_f(a_sb[:, k, :]),
                            func=AF.Square,
                            accum_out=sa0[:, k : k + 1],
                        )
                        nc.scalar.activation(
                            out=scr0,
                            in_=_f(b_sb[:, k, :]),
                            func=AF.Square,
                            accum_out=sb0[:, k : k + 1],
                        )
                    prod = work.tile([128, KT], F32, tag="prod")
                    nc.vector.tensor_mul(prod, sa0, sb0)
                    nc.scalar.activation(
                        out=prod, in_=prod, func=AF.Sqrt, bias=eps24
                    )
                    c_sb = work.tile([128, KT], F32, tag="c")
                    nc.vector.reciprocal(c_sb, prod)
                # row sumsq via PE ones-matmul over the transposed squares
                pss_c = ps.tile([1, 2 * T], F32, tag="tp")
                for qi, sq in enumerate((aTsq, bTsq)):
                    for k in range(KD):
                        nc.tensor.matmul(
                            pss_c[:, qi * T : (qi + 1) * T],
                            lhsT=ones[:, :],
                            rhs=sq[:, k, :],
                            start=(k == 0),
                            stop=(k == KD - 1),
                        )
                sc_row = work.tile([1, 2 * T], F32, tag="sc_row")
                nc.scalar.copy(sc_row, pss_c)
                psum_c = ps.tile([128, 2 * KT], F32, tag="tp")
                for j in range(2 * KT):
                    nc.tensor.transpose(
                        psum_c[:, j : j + 1],
                        sc_row[:, j * 128 : (j + 1) * 128],
                        identf[:1, :1],
                    )
                sc_sb = work.tile([128, 2 * KT], F32, tag="sc_sb")
                nc.scalar.copy(sc_sb, psum_c)
                prod = work.tile([128, KT], F32, tag="prod")
                nc.vector.tensor_mul(
                    prod, sc_sb[:, 0:KT], sc_sb[:, KT : 2 * KT]
                )
                nc.scalar.activation(
                    out=prod, in_=prod, func=AF.Sqrt, bias=eps24
                )
                c_sb = work.tile([128, KT], F32, tag="c")
                nc.vector.reciprocal(c_sb, prod)


                # ---- ac = a * c ----
                ac = work.tile([128, KT, D], F32R, tag="ac")
                for k in range(KT):
                    nc.gpsimd.tensor_scalar_mul(
                        ac[:, k, :], a_sb[:, k, :], c_sb[:, k : k + 1]
                    )

                # ---- alpha = b^T @ ac; colsumsq via ones-matmul ----
                alpha_sb = work.tile([128, KD, D], F32R, tag="alpha")
                ss_sb = work.tile([1, D], F32, tag="ss")
                psum_ss = ps.tile([1, D], F32, tag="sp")
                asqs = []
                for m in range(KD):
                    pal = ps.tile([128, D], F32, tag="alpha")
                    for k in range(KT):
                        nc.tensor.matmul(
                            pal,
                            lhsT=b_sb[:, k, m * 128 : (m + 1) * 128],
                            rhs=ac[:, k, :],
                            start=(k == 0),
                            stop=(k == KT - 1),
                        )
                    if m % 2 == 0:
                        nc.vector.tensor_copy(alpha_sb[:, m, :], pal)
                    else:
                        nc.scalar.copy(alpha_sb[:, m, :], pal)
                    asq = work.tile([128, D], F32R, tag=f"asq{m}")
                    nc.gpsimd.tensor_mul(
                        asq, alpha_sb[:, m, :], alpha_sb[:, m, :]
                    )
                    asqs.append(asq)
                for m in range(KD):
                    nc.tensor.matmul(
                        psum_ss,
                        lhsT=ones[:, :],
                        rhs=asqs[m][:, :],
                        start=(m == 0),
                        stop=(m == KD - 1),
                    )

                # ---- da = sum_e aT^2 * W2T, and rda = 1/sqrt(da) (early,
                # independent of hmean -> off the batch tail) ----
                pda = ps.tile([P, T], F32, tag="sp")
                for k in range(KD):
                    nc.tensor.matmul(
                        pda,
                        lhsT=w2t_sb[:, k, :],
                        rhs=aTsq[:, k, :],
                        start=(k == 0),
                        stop=(k == KD - 1),
                    )
                rda = work.tile([P, T], F32, tag="rda")
                nc.scalar.activation(
                    out=rda, in_=pda, func=AF.Sqrt, bias=eps30[:P]
                )
                nc.vector.reciprocal(rda, rda)

                # ---- s = 1/sqrt(colsumsq + eps) as [e,1] tiles ----
                sinv = work.tile([128, KD], F32, tag="sinv")
                nc.scalar.copy(ss_sb, psum_ss)
                psum_st = ps.tile([128, KD], F32, tag="sp")
                for m in range(KD):
                    nc.tensor.transpose(
                        psum_st[:, m : m + 1],
                        ss_sb[:, m * 128 : (m + 1) * 128],
                        identf[:1, :1],
                    )
                s_sq = work.tile([128, KD], F32, tag="s_sq")
                nc.scalar.activation(
                    out=s_sq, in_=psum_st, func=AF.Sqrt, bias=eps12
                )
                nc.vector.reciprocal(sinv, s_sq)

                # w2ts = s * W2T (weights for the num matmul)
                w2ts = work.tile([128, KD, P], F32R, tag="w2ts")
                for m in range(KD):
                    nc.gpsimd.tensor_scalar_mul(
                        w2ts[:, m, :], w2t_sb[:, m, :], sinv[:, m : m + 1]
                    )

                return dict(aT=aT, bT=bT, alpha_sb=alpha_sb, sinv=sinv,
                            w2ts=w2ts, rda=rda)

            def stage2(bi, st):
                aT, bT = st["aT"], st["bT"]
                alpha_sb, sinv, w2ts = st["alpha_sb"], st["sinv"], st["w2ts"]
                rda, w2t_loc = st["rda"], w2t_sb

                # ---- hT~[e, t] = sum_d alpha[d, e-tile] * bT[d, t] ----
                # consumed from PSUM: ahT = aT*hT~ (DVE), hTsq = (s*hT~)^2
                ahT = work.tile([128, KD, T], F32R, tag="ahT")
                hTsq = work.tile([128, KD, T], F32R, tag="hTsq")
                for m in range(KD):
                    ph = ps.tile([128, T], F32, tag="h")
                    for k in range(KD):
                        nc.tensor.matmul(
                            ph,
                            lhsT=alpha_sb[:, k, m * 128 : (m + 1) * 128],
                            rhs=bT[:, k, :],
                            start=(k == 0),
                            stop=(k == KD - 1),
                        )
                    nc.vector.tensor_mul(ahT[:, m, :], aT[:, m, :], _r(ph))
                    nc.scalar.activation(
                        out=hTsq[:, m, :],
                        in_=ph,
                        func=AF.Square,
                        scale=sinv[:, m : m + 1],
                    )

                # ---- dh and num matmuls, combine with rda/rdh ----
                pdh = ps.tile([P, T], F32, tag="h")
                for k in range(KD):
                    nc.tensor.matmul(
                        pdh,
                        lhsT=w2t_sb[:, k, :],
                        rhs=hTsq[:, k, :],
                        start=(k == 0),
                        stop=(k == KD - 1),
                    )
                rdh = work.tile([P, T], F32, tag="rdh")
                nc.scalar.activation(
                    out=rdh, in_=pdh, func=AF.Sqrt, bias=eps30[:P]
                )
                nc.vector.reciprocal(rdh, rdh)

                pnum = ps.tile([P, T], F32, tag="sp")
                for k in range(KD):
                    nc.tensor.matmul(
                        pnum,
                        lhsT=w2ts[:, k, :],
                        rhs=ahT[:, k, :],
                        start=(k == 0),
                        stop=(k == KD - 1),
                    )
                pp1 = work.tile([P, T], F32, tag="pp1")
                nc.vector.tensor_mul(pp1, pnum, rda)
                pp_sb = work.tile([P, T], F32R, tag="pp")
                nc.vector.tensor_mul(pp_sb, pp1, rdh)

                # ---- transpose persp [P, T] -> [t, p] and store ----
                out_sb = io.tile([128, KT, P], F32, tag="out")
                pso = ps.tile([128, KT * P], F32R, tag="sp")
                for j in range(KT):
                    nc.tensor.transpose(
                        pso[:, j * P : (j + 1) * P],
                        pp_sb[:, j * 128 : (j + 1) * 128],
                        ident[:P, :P],
                    )
                nc.scalar.copy(
                    out_sb, pso.rearrange("p (k q) -> p k q", k=KT)
                )
                nc.sync.dma_start(
                    out=out_dram[bi].rearrange("(k p) q -> p k q", p=128),
                    in_=out_sb,
                )

            def batch_body():
                st = {0: stage1(0)}
                for bi in range(BL):
                    if bi + 1 < BL:
                        st[bi + 1] = stage1(bi + 1)
                    stage2(bi, st.pop(bi))

            if reps > 1:
                with tc.For_i(0, reps, 1):
                    batch_body()
            else:
                batch_body()


def _build(reps=1):
    nc = bacc.Bacc("TRN2", target_bir_lowering=False, debug=False)
    a_ext = nc.declare_dram_parameter("inp_a", [BL, T, D], F32, isOutput=False)
    b_ext = nc.declare_dram_parameter("inp_b", [BL, T, D], F32, isOutput=False)
    w_ext = nc.declare_dram_parameter("W", [P, D], F32, isOutput=False)
    out_ext = nc.declare_dram_parameter("persp", [BL, T, P], F32, isOutput=True)
    _emit(nc, a_ext, b_ext, w_ext, out_ext, reps=reps)
    nc.compile()
    return nc


def get_nc(reps=1):
    if reps not in _NCS:
        _NCS[reps] = _build(reps=reps)
    return _NCS[reps]


def run_on_cores(inp_a, inp_b, W, reps=1, trace=False):
    from concourse.bass_utils import run_bass_kernel_spmd

    nc = get_nc(reps)
    inp_a = np.ascontiguousarray(inp_a, dtype=np.float32)
    inp_b = np.ascontiguousarray(inp_b, dtype=np.float32)
    W = np.ascontiguousarray(W, dtype=np.float32)
    in_maps = [
        {
            "inp_a": inp_a[i * BL : (i + 1) * BL],
            "inp_b": inp_b[i * BL : (i + 1) * BL],
            "W": W,
        }
        for i in range(N_CORES)
    ]
    return run_bass_kernel_spmd(nc, in_maps, list(range(N_CORES)), trace=trace)


def kernel(inp_a, inp_b, W):
    global LAST_RESULT
    res = run_on_cores(inp_a, inp_b, W, reps=1, trace=TRACE)
    LAST_RESULT = res
    persp = np.concatenate(
        [res.results[i]["persp"] for i in range(N_CORES)], axis=0
    )
    return (persp, persp)


# revision 65
# speedup vs baseline: 1.1435x; 1.0449x over previous
"""Trainium2 Bass kernel for AttentiveMatchingLayer.

Math (per batch):
  na = l2n(a, -1); nb = l2n(b, -1)
  alpha[d,e] = sum_t nb[t,d] * na[t,e]          (D x D)
  s[e] = 1/sqrt(max(sum_d alpha[d,e]^2, EPS))   (column l2n of alpha)
  h[t,e] = sum_d nb[t,d] * alpha[d,e] * s[e]
  persp[t,p] = sum_e aw*hw  with aw = l2n(a*W[p]), hw = l2n(h*W[p])
             = (sum_e a*h*W2) / (sqrt(sum_e a^2*W2) * sqrt(sum_e h^2*W2))
  where W2 = W^2. Row-scalings of a and b cancel inside persp, so:
    - alpha = b^T @ (a * c) with c[t] = 1/(||a_t|| * ||b_t||)
    - h~ = b @ alpha (raw b) is h up to a positive row scale -> usable
      directly; the alpha-column scale s is folded into the W2 weights of
      the num matmul and the ACT scale of the h^2 pass.

Sharding: pure data parallel, batch 32 -> 4 per core across 8 cores.
"""

import sys

import numpy as np

for _p in ("/opt/trn_rl_repo", "/root/.axon_site/_ro/trn_rl_repo"):
    if _p not in sys.path:
        sys.path.append(_p)

import concourse.bacc as bacc
import concourse.bass as bass
import concourse.tile as tile
from concourse import mybir
from concourse.masks import make_identity

N_CORES = 8
B, T, D, P = 32, 256, 512, 20
BL = B // N_CORES  # batches per core
KT = T // 128  # 2 t-tiles
KD = D // 128  # 4 d-tiles
F32 = mybir.dt.float32
F32R = mybir.dt.float32r  # full-rate fp32 matmul streaming (vs 4x for fp32)
EPS = 1e-12


def _r(ap):
    return ap.bitcast(F32R)


def _f(ap):
    return ap.bitcast(F32)
AF = mybir.ActivationFunctionType
OP = mybir.AluOpType

TRACE = False
LAST_RESULT = None

_NCS = {}


def _emit(nc, a_ext, b_ext, w_ext, out_ext, reps=1):
    a_dram = a_ext[:, :, :]
    b_dram = b_ext[:, :, :]
    w_dram = w_ext[:, :]
    out_dram = out_ext[:, :, :]

    with tile.TileContext(nc) as tc:
        import contextlib

        with contextlib.ExitStack() as ctx:
            const = ctx.enter_context(tc.tile_pool(name="const", bufs=1))
            io = ctx.enter_context(tc.tile_pool(name="io", bufs=3))
            work = ctx.enter_context(tc.tile_pool(name="work", bufs=3))
            ps = ctx.enter_context(tc.tile_pool(name="ps", bufs=2, space="PSUM"))

            # ---- constants ----
            identf = const.tile([128, 128], F32)
            make_identity(nc, identf)
            ident = const.tile([128, 128], F32R)
            nc.vector.tensor_copy(ident, identf)
            ones = const.tile([128, 1], F32R)
            nc.vector.memset(_f(ones), 1.0)
            eps12 = const.tile([128, 1], F32)
            nc.vector.memset(eps12, EPS)
            eps24 = const.tile([128, 1], F32)
            nc.vector.memset(eps24, 1e-24)
            eps30 = const.tile([128, 1], F32)
            nc.vector.memset(eps30, 1e-30)

            # W2T[d, p] = W[p, d]^2 transposed
            w_sb = const.tile([P, D], F32)
            nc.sync.dma_start(out=w_sb, in_=w_dram)
            w2_sb = const.tile([P, D], F32R)
            nc.scalar.square(w2_sb, w_sb)
            w2t_sb = const.tile([128, KD, P], F32R)
            psum_w2t = ps.tile([128, KD * P], F32R, tag="sp")
            for m in range(KD):
                nc.tensor.transpose(
                    psum_w2t[:, m * P : (m + 1) * P],
                    w2_sb[:, m * 128 : (m + 1) * 128],
                    ident[:P, :P],
                )
            nc.scalar.copy(w2t_sb, psum_w2t.rearrange("p (m q) -> p m q", m=KD))

            def stage1(bi):
                    # ---- load a, b ----
                a_sb = io.tile([128, KT, D], F32R, tag="a")
                b_sb = io.tile([128, KT, D], F32R, tag="b")
                for k in range(KT):
                    nc.sync.dma_start(
                        out=a_sb[:, k, :],
                        in_=_r(a_dram[bi, k * 128 : (k + 1) * 128, :]),
                    )
                    nc.sync.dma_start(
                        out=b_sb[:, k, :],
                        in_=_r(b_dram[bi, k * 128 : (k + 1) * 128, :]),
                    )

                # ---- transposes: aT[d, t], bT[d, t] (PE) ----
                aT = work.tile([128, KD, T], F32R, tag="aT")
                bT = work.tile([128, KD, T], F32R, tag="bT")
                aTsq = work.tile([128, KD, T], F32R, tag="aTsq")
                bTsq = work.tile([128, KD, T], F32R, tag="bTsq")
                for si, (src, dstv, sq) in enumerate(
                    ((a_sb, aT, aTsq), (b_sb, bT, bTsq))
                ):
                    for k in range(KT):
                        ptp = ps.tile([128, 512], F32R, tag="tp")
                        for m in range(KD):
                            nc.tensor.transpose(
                                ptp[:, m * 128 : (m + 1) * 128],
                                src[:, k, m * 128 : (m + 1) * 128],
                                ident,
                            )
                        ptpv = ptp.rearrange("p (m q) -> p m q", m=KD)
                        dst = dstv[:, :, k * 128 : (k + 1) * 128]
                        nc.vector.tensor_copy(dst, ptpv)
                        sqdst = sq[:, :, k * 128 : (k + 1) * 128]
                        nc.gpsimd.tensor_mul(sqdst, dst, dst)

                # ---- c = 1/(|a_t| * |b_t|) ----
                if bi == 0:
                    # fill-phase fast path: ACT square+accum straight off
                    # a_sb/b_sb, concurrent with the PE transposes
                    sa0 = work.tile([128, KT], F32, tag="sa0")
                    sb0 = work.tile([128, KT], F32, tag="sb0")
                    scr0 = work.tile([128, D], F32, tag="scr0")
                    for k in range(KT):
                        nc.scalar.activation(
                            out=scr0,
                            in_=_f(a_sb[:, k, :]),
                            func=AF.Square,
                            accum_out=sa0[:, k : k + 1],
                        )
                        nc.scalar.activation(
                            out=scr0,
                            in_=_f(b_sb[:, k, :]),
                            func=AF.Square,
                            accum_out=sb0[:, k : k + 1],
                        )
                    prod = work.tile([128, KT], F32, tag="prod")
                    nc.vector.tensor_mul(prod, sa0, sb0)
                    nc.scalar.activation(
                        out=prod, in_=prod, func=AF.Sqrt, bias=eps24
                    )
                    c_sb = work.tile([128, KT], F32, tag="c")
                    nc.vector.reciprocal(c_sb, prod)
                # row sumsq via PE ones-matmul over the transposed squares
                pss_c = ps.tile([1, 2 * T], F32, tag="tp")
                for qi, sq in enumerate((aTsq, bTsq)):
                    for k in range(KD):
                        nc.tensor.matmul(
                            pss_c[:, qi * T : (qi + 1) * T],
                            lhsT=ones[:, :],
                            rhs=sq[:, k, :],
                            start=(k == 0),
                            stop=(k == KD - 1),
                        )
                sc_row = work.tile([1, 2 * T], F32, tag="sc_row")
                nc.scalar.copy(sc_row, pss_c)
                psum_c = ps.tile([128, 2 * KT], F32, tag="tp")
                for j in range(2 * KT):
                    nc.tensor.transpose(
                        psum_c[:, j : j + 1],
                        sc_row[:, j * 128 : (j + 1) * 128],
                        identf[:1, :1],
                    )
                sc_sb = work.tile([128, 2 * KT], F32, tag="sc_sb")
                nc.scalar.copy(sc_sb, psum_c)
                prod = work.tile([128, KT], F32, tag="prod")
                nc.vector.tensor_mul(
                    prod, sc_sb[:, 0:KT], sc_sb[:, KT : 2 * KT]
                )
                nc.scalar.activation(
                    out=prod, in_=prod, func=AF.Sqrt, bias=eps24
                )
                c_sb = work.tile([128, KT], F32, tag="c")
                nc.vector.reciprocal(c_sb, prod)


                # ---- ac = a * c ----
                ac = work.tile([128, KT, D], F32R, tag="ac")
                for k in range(KT):
                    nc.gpsimd.tensor_scalar_mul(
                        ac[:, k, :], a_sb[:, k, :], c_sb[:, k : k + 1]
                    )

                # ---- alpha = b^T @ ac; colsumsq via ones-matmul ----
                alpha_sb = work.tile([128, KD, D], F32R, tag="alpha")
                ss_sb = work.tile([1, D], F32, tag="ss")
                psum_ss = ps.tile([1, D], F32, tag="sp")
                asqs = []
                for m in range(KD):
                    pal = ps.tile([128, D], F32, tag="alpha")
                    for k in range(KT):
                        nc.tensor.matmul(
                            pal,
                            lhsT=b_sb[:, k, m * 128 : (m + 1) * 128],
                            rhs=ac[:, k, :],
                            start=(k == 0),
                            stop=(k == KT - 1),
                        )
                    if m % 2 == 0:
                        nc.vector.tensor_copy(alpha_sb[:, m, :], pal)
                    else:
                        nc.scalar.copy(alpha_sb[:, m, :], pal)
                    asq = work.tile([128, D], F32R, tag=f"asq{m}")
                    nc.gpsimd.tensor_mul(
                        asq, alpha_sb[:, m, :], alpha_sb[:, m, :]
                    )
                    asqs.append(asq)
                for m in range(KD):
                    nc.tensor.matmul(
                        psum_ss,
                        lhsT=ones[:, :],
                        rhs=asqs[m][:, :],
                        start=(m == 0),
                        stop=(m == KD - 1),
                    )

                # ---- da = sum_e aT^2 * W2T, and rda = 1/sqrt(da) (early,
                # independent of hmean -> off the batch tail) ----
                pda = ps.tile([P, T], F32, tag="sp")
                for k in range(KD):
                    nc.tensor.matmul(
                        pda,
                        lhsT=w2t_sb[:, k, :],
                        rhs=aTsq[:, k, :],
                        start=(k == 0),
                        stop=(k == KD - 1),
                    )
                rda = work.tile([P, T], F32, tag="rda")
                nc.scalar.activation(
                    out=rda, in_=pda, func=AF.Sqrt, bias=eps30[:P]
                )
                nc.vector.reciprocal(rda, rda)

                # ---- s = 1/sqrt(colsumsq + eps) as [e,1] tiles ----
                sinv = work.tile([128, KD], F32, tag="sinv")
                nc.scalar.copy(ss_sb, psum_ss)
                psum_st = ps.tile([128, KD], F32, tag="sp")
                for m in range(KD):
                    nc.tensor.transpose(
                        psum_st[:, m : m + 1],
                        ss_sb[:, m * 128 : (m + 1) * 128],
                        identf[:1, :1],
                    )
                s_sq = work.tile([128, KD], F32, tag="s_sq")
                nc.scalar.activation(
                    out=s_sq, in_=psum_st, func=AF.Sqrt, bias=eps12
                )
                nc.vector.reciprocal(sinv, s_sq)

                # w2ts = s * W2T (weights for the num matmul)
                w2ts = work.tile([128, KD, P], F32R, tag="w2ts")
                for m in range(KD):
                    nc.gpsimd.tensor_scalar_mul(
                        w2ts[:, m, :], w2t_sb[:, m, :], sinv[:, m : m + 1]
                    )

                return dict(aT=aT, bT=bT, alpha_sb=alpha_sb, sinv=sinv,
                            w2ts=w2ts, rda=rda)

            def stage2(bi, st):
                aT, bT = st["aT"], st["bT"]
                alpha_sb, sinv, w2ts = st["alpha_sb"], st["sinv"], st["w2ts"]
                rda, w2t_loc = st["rda"], w2t_sb

                # ---- hT~[e, t] = sum_d alpha[d, e-tile] * bT[d, t] ----
                # consumed from PSUM: ahT = aT*hT~ (DVE), hTsq = (s*hT~)^2
                ahT = work.tile([128, KD, T], F32R, tag="ahT")
                hTsq = work.tile([128, KD, T], F32R, tag="hTsq")
                for m in range(KD):
                    ph = ps.tile([128, T], F32, tag="h")
                    for k in range(KD):
                        nc.tensor.matmul(
                            ph,
                            lhsT=alpha_sb[:, k, m * 128 : (m + 1) * 128],
                            rhs=bT[:, k, :],
                            start=(k == 0),
                            stop=(k == KD - 1),
                        )
                    nc.vector.tensor_mul(ahT[:, m, :], aT[:, m, :], _r(ph))
                    nc.scalar.activation(
                        out=hTsq[:, m, :],
                        in_=ph,
                        func=AF.Square,
                        scale=sinv[:, m : m + 1],
                    )

                # ---- dh and num matmuls, combine with rda/rdh ----
                pdh = ps.tile([P, T], F32, tag="h")
                for k in range(KD):
                    nc.tensor.matmul(
                        pdh,
                        lhsT=w2t_sb[:, k, :],
                        rhs=hTsq[:, k, :],
                        start=(k == 0),
                        stop=(k == KD - 1),
                    )
                rdh = work.tile([P, T], F32, tag="rdh")
                nc.scalar.activation(
                    out=rdh, in_=pdh, func=AF.Sqrt, bias=eps30[:P]
                )
                nc.vector.reciprocal(rdh, rdh)

                pnum = ps.tile([P, T], F32, tag="sp")
                for k in range(KD):
                    nc.tensor.matmul(
                        pnum,
                        lhsT=w2ts[:, k, :],
                        rhs=ahT[:, k, :],
                        start=(k == 0),
                        stop=(k == KD - 1),
                    )
                pp1 = work.tile([P, T], F32, tag="pp1")
                nc.vector.tensor_mul(pp1, pnum, rda)
                pp_sb = work.tile([P, T], F32R, tag="pp")
                nc.vector.tensor_mul(pp_sb, pp1, rdh)

                # ---- transpose persp [P, T] -> [t, p] and store ----
                out_sb = io.tile([128, KT, P], F32, tag="out")
                pso = ps.tile([128, KT * P], F32R, tag="sp")
                for j in range(KT):
                    nc.tensor.transpose(
                        pso[:, j * P : (j + 1) * P],
                        pp_sb[:, j * 128 : (j + 1) * 128],
                        ident[:P, :P],
                    )
                nc.scalar.copy(
                    out_sb, pso.rearrange("p (k q) -> p k q", k=KT)
                )
                nc.sync.dma_start(
                    out=out_dram[bi].rearrange("(k p) q -> p k q", p=128),
                    in_=out_sb,
                )

            def batch_body():
                st = {0: stage1(0)}
                for bi in range(BL):
                    if bi + 1 < BL:
                        st[bi + 1] = stage1(bi + 1)
                    stage2(bi, st.pop(bi))

            if reps > 1:
                with tc.For_i(0, reps, 1):
                    batch_body()
            else:
                batch_body()


def _build(reps=1):
    nc = bacc.Bacc("TRN2", target_bir_lowering=False, debug=False)
    a_ext = nc.declare_dram_parameter("inp_a", [BL, T, D], F32, isOutput=False)
    b_ext = nc.declare_dram_parameter("inp_b", [BL, T, D], F32, isOutput=False)
    w_ext = nc.declare_dram_parameter("W", [P, D], F32, isOutput=False)
    out_ext = nc.declare_dram_parameter("persp", [BL, T, P], F32, isOutput=True)
    _emit(nc, a_ext, b_ext, w_ext, out_ext, reps=reps)
    nc.compile()
    return nc


def get_nc(reps=1):
    if reps not in _NCS:
        _NCS[reps] = _build(reps=reps)
    return _NCS[reps]


def run_on_cores(inp_a, inp_b, W, reps=1, trace=False):
    from concourse.bass_utils import run_bass_kernel_spmd

    nc = get_nc(reps)
    inp_a = np.ascontiguousarray(inp_a, dtype=np.float32)
    inp_b = np.ascontiguousarray(inp_b, dtype=np.float32)
    W = np.ascontiguousarray(W, dtype=np.float32)
    in_maps = [
        {
            "inp_a": inp_a[i * BL : (i + 1) * BL],
            "inp_b": inp_b[i * BL : (i + 1) * BL],
            "W": W,
        }
        for i in range(N_CORES)
    ]
    return run_bass_kernel_spmd(nc, in_maps, list(range(N_CORES)), trace=trace)


def kernel(inp_a, inp_b, W):
    global LAST_RESULT
    res = run_on_cores(inp_a, inp_b, W, reps=1, trace=TRACE)
    LAST_RESULT = res
    persp = np.concatenate(
        [res.results[i]["persp"] for i in range(N_CORES)], axis=0
    )
    return (persp, persp)
